# revision 1
# baseline (speedup 1.0000x reference)
"""Trainium2 Bass kernel for nn_PhaseLoss: three wrapped phase-loss terms.

loss = mean(unwrap(d)) + mean(unwrap(shift_diff_freq(d))) + mean(unwrap(shift_diff_time(d)))
with d = angle(ref) - angle(est), unwrap(x) = |x - 2pi*round(x/2pi)|.

Pure data parallel over batch (8 cores x 4 batches); per core, partition
dim = freq (4 tiles x 128 f-rows), free dim = (batch, time).

Per unit, fully pipelined (no ACT table switches: atan/abs/copy share one
table set):
  - load: host pre-stacks the four inputs batch-major [b, 4, f, t]; ONE
    gpsimd cast-DMA per unit lands all four planes as bf16 (no convert
    instructions, 4x fewer SWDGE emissions on the Pool queue)
  - DVE: per-batch A|B = (rr*gr | ri*gi), C = ri*gr, D = rr*gi (bf16 2x),
    di = C - D
  - TensorE: dr = I@A + I@B (accumulating identity matmuls -> PSUM f32).
    di deliberately stays OFF the PE: a third PSUM allocation per unit
    (over 2 rotating buffers) serializes consecutive units and starves
    the DMA pipeline (-40us/iter measured when removed).
  - DVE custom QDIV: q = di/(dr+eps), dr read from PSUM (the bit-NOT
    reciprocal seed is odd in the sign bit, so one fused signed recip +
    Newton + multiply; eps not bf16-representable => dr+eps != 0)
  - ScalarE: t = arctan(q); DVE custom WFIX: w = dr<0 ? t - sign(t)*pi : t
  - gd: TensorE delta = (S-I)@w -> PSUM, DVE custom WRAPABS accumulates
    min(|delta|, 2pi-|delta|)
  - ptd: DVE custom WRAPDIFF on time-adjacent w pairs (strided)
  - ip: ScalarE Abs with accum_out
  - last f-tile split into two half-batch units (shorter post-last-DMA
    dependency tail on the one-shot measurement)

Host covers the boundary terms from raw inputs (~0.5% of elements):
f=512 row (all terms), gd rows f in {0, 128, 256, 384}, ptd t=0 column.
"""

import numpy as np

B, F, T = 32, 513, 512
NCORES = 8
BPC = B // NCORES  # 4
NTILES = 4  # 4 x 128 partitions = f in [0, 512)
FREE = BPC * T  # 2048
PI = float(np.pi)
TWO_PI = 2.0 * PI
EPS_GUARD = 1e-30
IN_NAMES = ("spec_est_real", "spec_est_imag", "spec_ref_real", "spec_ref_imag")

_CACHE: dict = {}
TRACE = False
LAST_RESULT = None
REPEAT = 1

# perf toggles
CAST_DMA = True   # gpsimd cast f32->bf16 during load (else sync + ACT converts)
PE_SUMS = True    # dr/di via TensorE identity matmuls (else DVE add/sub)
GD_MATMUL = True  # freq shift-diff via TensorE (else SBUF-SBUF shift DMA)
RAW_BUFS = 6
POOL_MULC = False  # one product mul on GpSimd instead of DVE
# batch-splits per f-tile, e.g. (1,1,1,2) halves the last tile to shorten
# the final dependency-chain tail after the last DMA
SPLITS = (1, 1, 1, 2)
SKIP_COMPUTE = False  # diagnostic: issue loads only
REUSE_RAW = False     # diagnostic: load one tile set, compute all units on it
PE_DI = False         # di on DVE: a 3rd PSUM alloc per unit serializes the pipeline
PE_DR = True          # dr = A + B on TensorE (else DVE add)
DR_COPY = False       # copy dr PSUM->SBUF early (ScalarE) to free its banks
PSUM_CHUNK = 2048     # PSUM tile width (PSUM alloc granularity blocks finer)
P2_BUFS = 2           # di/q/t/w working-tile rotation depth
PACKED_LOAD = True   # inputs pre-stacked [4,b,f,t]; ONE cast-DMA per unit
POOL_SUB = False      # di = C - D on GpSimd instead of DVE
SPLIT_WIDEMUL = True # per-batch AB muls: PE dr starts after first batch
SPLIT_CD = True      # per-batch mulC/mulD/di-sub/qdiv chain


# --------------------------------------------------------------------------- #
# custom DVE ops
# --------------------------------------------------------------------------- #
def _get_ops2():
    if "ops2" in _CACHE:
        return _CACHE["ops2"]
    import concourse.dve_ops as dve_ops
    from concourse.dve_ops import DveOp
    from concourse.dve_spec import (
        Bin,
        Spec,
        Src0,
        Src1,
        Zero,
        C0,
        C1,
        C2,
        _has_src1,
        lower,
        maxx,
        minn,
        select,
    )
    from concourse.dve_uop import AluOp, DveOpSpec
    from operator import add

    def mk(name, spec, subdim=False):
        for op in dve_ops.OPS:
            if op.name == name:
                return op
        shas = {}
        for ver in ("v3", "v4"):
            try:
                shas[ver] = DveOpSpec(
                    name=name, uops=lower(spec, ver=ver), rd1_en=_has_src1(spec)
                ).sha(ver)
            except Exception:
                pass
        op = DveOp(name, spec, subdim=subdim, uops_sha=shas)
        dve_ops.OPS.append(op)
        dve_ops._SUB_OPCODE_FOR_NAME[op.name] = (
            dve_ops._CUSTOM_DVE_ROW_BASE + len(dve_ops.OPS) - 1
        )
        dve_ops.CUSTOM_DVE_SPECS[op.name] = op.spec
        return op

    # signed guarded reciprocal: out ~= 1/(Src0 + C2). The bit-NOT seed is
    # odd in the sign bit (NOT(-x) = -NOT(x) as float bits), so the same
    # seed + Newton works for both signs directly; C2 shifts 0 -> eps.
    _g = Src0 + C2
    _nx = Bin(AluOp.BITWISE_NOT, _g, _g)
    _y0 = _nx * C0
    _y1 = _y0 * (C1 - _g * _y0)

    def _rs_ref(in0, in1, s0, s1, imm2):
        g = (in0.astype(np.float32) + np.float32(imm2)).astype(np.float32)
        nx = (~g.view(np.int32)).view(np.float32)
        y0 = (nx * np.float32(s0)).astype(np.float32)
        return (y0 * (np.float32(s1) - g * y0)).astype(np.float32)

    recips = mk("PL2_RECIPS", Spec(body=_y1, reference=_rs_ref))

    # fused q = Src0 / (Src1 + C2): signed NOT-seed reciprocal of dr (Src1)
    # times di (Src0) in one pass. Src1 may live in PSUM.
    _g2 = Src1 + C2
    _nx2 = Bin(AluOp.BITWISE_NOT, _g2, _g2)
    _z0 = _nx2 * C0
    _z1 = _z0 * (C1 - _g2 * _z0)

    def _qd_ref(in0, in1, s0, s1, imm2):
        g = (in1.astype(np.float32) + np.float32(imm2)).astype(np.float32)
        nx = (~g.view(np.int32)).view(np.float32)
        y0 = (nx * np.float32(s0)).astype(np.float32)
        y1 = (y0 * (np.float32(s1) - g * y0)).astype(np.float32)
        return (in0.astype(np.float32) * y1).astype(np.float32)

    qdiv = mk("PL2_QDIV", Spec(body=Src0 * _z1, reference=_qd_ref))

    # w = dr<0 ? t - sign(t)*pi : t    (Src0=t, Src1=dr, C0=pi)
    _sp = select(Src0 < Zero, Zero - C0, C0)

    def _wf_ref(in0, in1, s0, s1, imm2):
        t = in0.astype(np.float32)
        sp = np.where(t < 0, -np.float32(s0), np.float32(s0)).astype(np.float32)
        return np.where(in1.astype(np.float32) < 0, t - sp, t).astype(np.float32)

    wfix2 = mk(
        "PL2_WFIX",
        Spec(body=select(Src1 < Zero, Src0 - _sp, Src0), reference=_wf_ref),
    )

    # gd: out = min(|x|, 2pi-|x|), accum sum   (Src0 = x = shifted diff)
    _a1 = Bin(AluOp.ABSOLUTE_VALUE, Src0, Src0)

    def _wa_ref(in0, in1, s0, s1, imm2):
        a = np.abs(in0.astype(np.float32))
        b = np.minimum(a, np.float32(s0) - a).astype(np.float32)
        return b, b.reshape(b.shape[0], -1).sum(axis=-1, keepdims=True)

    wrapabs = mk(
        "PL2_WRAPABS",
        Spec(body=minn(_a1, C0 - _a1), accum=add, accum_init=Zero,
             reference=_wa_ref),
    )

    # ptd: out = min(|a-b|, 2pi-|a-b|), accum sum
    _d = Bin(AluOp.ABSOLUTE_DIFF, Src0, Src1)

    def _wd_ref(in0, in1, s0, s1, imm2):
        ad = np.abs(in0.astype(np.float32) - in1.astype(np.float32))
        b = np.minimum(ad, s0 - ad).astype(np.float32)
        return b, b.reshape(b.shape[0], -1).sum(axis=-1, keepdims=True)

    wrapdiff = mk(
        "PL2_WRAPDIFF",
        Spec(body=minn(_d, C0 - _d), accum=add, accum_init=Zero,
             reference=_wd_ref),
    )

    _CACHE["ops2"] = (recips, wfix2, wrapabs, wrapdiff, qdiv)
    return _CACHE["ops2"]


def shift_mats() -> np.ndarray:
    """[128, 384] f32: I | -I | (S-I)^T  (lhsT layout, out = lhsT.T @ rhs)."""
    ident = np.eye(128, dtype=np.float32)
    shift = np.zeros((128, 128), dtype=np.float32)
    # out[p] = w[p-1] - w[p] for p>=1; out[0] = 0
    shift[np.arange(127), np.arange(1, 128)] = 1.0
    shift[np.arange(1, 128), np.arange(1, 128)] = -1.0
    return np.concatenate([ident, -ident, shift], axis=1)


# --------------------------------------------------------------------------- #
# bass program (identical on all 8 cores)
# --------------------------------------------------------------------------- #
def _build_bass():
    if "nc" in _CACHE:
        return _CACHE["nc"]
    import concourse.bacc as bacc
    import concourse.tile as tile
    from concourse import mybir

    recips, wfix2, wrapabs, wrapdiff, qdiv = _get_ops2()
    from concourse.dve_ops import RECIP_APPROX_FAST_CONSTS as RC

    dt = mybir.dt
    AF = mybir.ActivationFunctionType

    nc = bacc.Bacc("TRN2", name="phase_loss2")
    if PACKED_LOAD:
        all_d = nc.dram_tensor("spec_all", [BPC, 4, F, T], dt.float32,
                               kind="ExternalInput")
    else:
        ins = {
            n: nc.dram_tensor(n, [BPC, F, T], dt.float32, kind="ExternalInput")
            for n in IN_NAMES
        }
    mats_d = nc.dram_tensor("shift_mats", [128, 384], dt.float32,
                            kind="ExternalInput")
    out_d = nc.dram_tensor("partials", [128, 48], dt.float32,
                           kind="ExternalOutput")

    FR = FREE  # 2048

    with tile.TileContext(nc) as tc:
        with (
            tc.tile_pool(name="raw", bufs=RAW_BUFS) as p_raw,
            tc.tile_pool(name="raw2", bufs=2) as p_raw2,
            tc.tile_pool(name="prod", bufs=2) as p_prod,
            tc.tile_pool(name="pers", bufs=2) as p_pers,
            tc.tile_pool(name="p2", bufs=P2_BUFS) as p_2,
            tc.tile_pool(name="junk", bufs=2) as p_junk,
            tc.tile_pool(name="consts", bufs=1) as p_c,
            tc.tile_pool(name="accp", bufs=1) as p_acc,
            tc.psum_pool(name="ps", bufs=2 * (FREE // PSUM_CHUNK)) as p_ps,
        ):
            acc = p_acc.tile([128, 48], dt.float32, tag="acc")
            nc.vector.memset(acc[:], 0.0)
            # mats via HWDGE + ACT convert: keeps the gpsimd queue free
            # for the first raw-tile cast-DMAs (ramp)
            mats = p_c.tile([128, 384], dt.bfloat16, tag="mats")
            matf = p_c.tile([128, 384], dt.float32, tag="matf")
            nc.sync.dma_start(matf[:], mats_d[:, :])
            nc.scalar.copy(mats[:], matf[:])
            ident = mats[:, 0:128]
            nident = mats[:, 128:256]
            shiftm = mats[:, 256:384]

            for _rep in range(REPEAT):

                raw_stash = {}

                def phase1(k, h0, nb, ui):
                    W = nb * T
                    # ---- loads: U=(rr|ri), V=(gr|gi), each [128, 2, nb, t]
                    pr = p_raw if nb == BPC else p_raw2
                    if PACKED_LOAD:
                        # one cast-DMA per unit; per-partition layout is
                        # b-major: [b][gr,gi,rr,ri][t]
                        UV = pr.tile([128, nb, 4, T], dt.bfloat16,
                                     tag=f"UV{nb}")
                        src = all_d[
                            h0 : h0 + nb, :, 128 * k : 128 * (k + 1), :
                        ].rearrange("b c f t -> f (b c) t")
                        nc.gpsimd.dma_start(
                            UV[:].rearrange("p b c t -> p (b c) t"), src
                        )
                        if SKIP_COMPUTE:
                            return
                        # 3D views (free dims [nb, .]); inner step 1
                        Uv = UV[:, :, 2:4, :].rearrange("p b h t -> p b (h t)")
                        Vv = UV[:, :, 0:2, :].rearrange("p b h t -> p b (h t)")
                        Ur = UV[:, :, 2, :]
                        Ui = UV[:, :, 3, :]
                        Vr = UV[:, :, 0, :]
                        Vi = UV[:, :, 1, :]
                        return_views = (Uv, Vv, Ur, Ui, Vr, Vi)
                    elif REUSE_RAW and "uv" in raw_stash:
                        U, V = raw_stash["uv"]
                        loads = []
                    else:
                        U = pr.tile([128, 2, nb, T], dt.bfloat16, tag=f"U{nb}")
                        V = pr.tile([128, 2, nb, T], dt.bfloat16, tag=f"V{nb}")
                        raw_stash["uv"] = (U, V)
                        loads = []
                    if PACKED_LOAD:
                        loads = []
                    else:
                        loads = (
                            []
                            if (REUSE_RAW and raw_stash.get("done"))
                            else [
                                (U, 0, "spec_ref_real"),
                                (U, 1, "spec_ref_imag"),
                                (V, 0, "spec_est_real"),
                                (V, 1, "spec_est_imag"),
                            ]
                        )
                        if REUSE_RAW:
                            raw_stash["done"] = True
                    for dst, half, name in loads:
                        src = ins[name][
                            h0 : h0 + nb, 128 * k : 128 * (k + 1), :
                        ].rearrange("b f t -> f b t")
                        if CAST_DMA:
                            nc.gpsimd.dma_start(dst[:, half], src)
                        else:
                            r = p_raw.tile([128, nb, T], dt.float32,
                                           tag=f"rf{nb}")
                            nc.sync.dma_start(r[:], src)
                            nc.scalar.copy(
                                dst[:, half].rearrange("p b t -> p (b t)"),
                                r[:].rearrange("p b t -> p (b t)"),
                            )
                    if not PACKED_LOAD:
                        if SKIP_COMPUTE:
                            return
                        Uv = U[:].rearrange("p h b t -> p (h b t)")
                        Vv = V[:].rearrange("p h b t -> p (h b t)")
                        Ur = U[:, 0].rearrange("p b t -> p (b t)")
                        Ui = U[:, 1].rearrange("p b t -> p (b t)")
                        Vr = V[:, 0].rearrange("p b t -> p (b t)")
                        Vi = V[:, 1].rearrange("p b t -> p (b t)")
                    else:
                        Uv, Vv, Ur, Ui, Vr, Vi = return_views

                    # ---- products
                    AB = p_prod.tile([128, 2 * FR], dt.bfloat16, tag="AB")
                    Cf = p_pers.tile([128, FR], dt.bfloat16, tag="C")
                    Ct = Cf[:, 0:W]
                    Df = p_pers.tile([128, FR], dt.bfloat16, tag="D")
                    Dt = Df[:, 0:W]
                    if PACKED_LOAD:
                        # AB is b-interleaved: per b, [A_b (T) | B_b (T)]
                        AB3 = AB[:, 0 : 2 * W].rearrange(
                            "p (b x) -> p b x", b=nb
                        )
                        if SPLIT_WIDEMUL:
                            for bb in range(nb):
                                nc.vector.tensor_mul(
                                    AB3[:, bb], Uv[:, bb], Vv[:, bb]
                                )
                        else:
                            nc.vector.tensor_mul(AB3, Uv, Vv)
                        Ct3 = Cf[:, 0:W].rearrange("p (b t) -> p b t", b=nb)
                        Dt3 = Df[:, 0:W].rearrange("p (b t) -> p b t", b=nb)
                        if SPLIT_CD:
                            for bb in range(nb):
                                nc.vector.tensor_mul(
                                    Ct3[:, bb], Ui[:, bb], Vr[:, bb]
                                )
                                nc.vector.tensor_mul(
                                    Dt3[:, bb], Ur[:, bb], Vi[:, bb]
                                )
                        elif POOL_MULC:
                            nc.gpsimd.tensor_mul(Ct3, Ui, Vr)  # ri*gr
                            nc.vector.tensor_mul(Dt3, Ur, Vi)
                        else:
                            nc.vector.tensor_mul(Ct3, Ui, Vr)
                            nc.vector.tensor_mul(Dt3, Ur, Vi)  # rr*gi
                    else:
                        nc.vector.tensor_mul(AB[:, 0 : 2 * W], Uv, Vv)
                        if POOL_MULC:
                            nc.gpsimd.tensor_mul(Ct, Ui, Vr)  # ri*gr
                        else:
                            nc.vector.tensor_mul(Ct, Ui, Vr)
                        nc.vector.tensor_mul(Dt, Ur, Vi)  # rr*gi

                    # dr = A + B, di = C - D via accumulating identity
                    # matmuls; both stay in PSUM (f32).
                    nmm = W // 512
                    CW = min(PSUM_CHUNK, W)
                    nck = W // CW
                    if PE_DR:
                        # dr in PSUM, one pool tile per CW-wide chunk
                        dr_chunks = []
                        for c in range(nck):
                            drp = p_ps.tile([128, CW], dt.float32, tag="ps")
                            for j in range(CW // 512):
                                sl = slice(512 * j, 512 * (j + 1))
                                g0 = c * CW + 512 * j
                                if PACKED_LOAD:
                                    a0 = 2 * g0
                                    b0 = 2 * g0 + 512
                                else:
                                    a0 = g0
                                    b0 = W + g0
                                nc.tensor.matmul(drp[:, sl], ident,
                                                 AB[:, a0 : a0 + 512],
                                                 start=True, stop=False)
                                nc.tensor.matmul(
                                    drp[:, sl], ident,
                                    AB[:, b0 : b0 + 512],
                                    start=False, stop=True,
                                )
                            dr_chunks.append(drp)
                        if DR_COPY:
                            dr_f = p_2.tile([128, FR], dt.bfloat16, tag="drsb")
                            for c in range(nck):
                                nc.scalar.copy(
                                    dr_f[:, c * CW : (c + 1) * CW],
                                    dr_chunks[c][:],
                                )
                            dr_aps = [
                                dr_f[:, c * CW : (c + 1) * CW]
                                for c in range(nck)
                            ]
                        else:
                            dr_aps = [drp[:] for drp in dr_chunks]
                    else:
                        dr_f = p_2.tile([128, FR], dt.bfloat16, tag="drsb")
                        nc.vector.tensor_add(
                            dr_f[:, 0:W], AB[:, 0:W], AB[:, W : 2 * W]
                        )
                        dr_aps = [
                            dr_f[:, c * CW : (c + 1) * CW] for c in range(nck)
                        ]
                    di_f = p_2.tile([128, FR], dt.bfloat16, tag="di")
                    di_sb = di_f[:, 0:W]
                    if PE_DI:
                        dip = p_ps.tile([128, FR], dt.float32, tag="ps")
                        for j in range(nmm):
                            sl = slice(512 * j, 512 * (j + 1))
                            nc.tensor.matmul(dip[:, sl], ident, Cf[:, sl],
                                             start=True, stop=False)
                            nc.tensor.matmul(dip[:, sl], nident, Df[:, sl],
                                             start=False, stop=True)
                        nc.scalar.copy(di_sb, dip[:, 0:W])
                    elif SPLIT_CD and PACKED_LOAD:
                        di3 = di_f[:, 0:W].rearrange(
                            "p (b t) -> p b t", b=nb
                        )
                        Ct3r = Cf[:, 0:W].rearrange("p (b t) -> p b t", b=nb)
                        Dt3r = Df[:, 0:W].rearrange("p (b t) -> p b t", b=nb)
                        for bb in range(nb):
                            nc.vector.tensor_sub(
                                di3[:, bb], Ct3r[:, bb], Dt3r[:, bb]
                            )
                    elif POOL_SUB:
                        nc.gpsimd.tensor_sub(di_sb, Ct, Dt)
                    else:
                        nc.vector.tensor_sub(di_sb, Ct, Dt)
                    # q = di / (dr + eps), fused NOT-seed reciprocal; eps is
                    # not bf16-representable so dr+eps != 0.
                    q_f = p_2.tile([128, FR], dt.bfloat16, tag="q")
                    q = q_f[:, 0:W]
                    if SPLIT_CD and PACKED_LOAD and nck == 1:
                        for bb in range(nb):
                            bs = slice(bb * T, (bb + 1) * T)
                            nc.vector._custom_dve(
                                qdiv, out=q_f[:, bs], in0=di_f[:, bs],
                                in1=dr_chunks[0][:, bs],
                                s0=RC["s0"], s1=RC["s1"], imm2=1.0000001e-6,
                            )
                    else:
                        for c in range(nck):
                            cs = slice(c * CW, (c + 1) * CW)
                            nc.vector._custom_dve(
                                qdiv, out=q_f[:, cs], in0=di_f[:, cs],
                                in1=dr_aps[c],
                                s0=RC["s0"], s1=RC["s1"], imm2=1.0000001e-6,
                            )
                    t_f = p_2.tile([128, FR], dt.bfloat16, tag="t")
                    t = t_f[:, 0:W]
                    nc.scalar.activation(t, q, AF.Arctan)
                    w_f = p_2.tile([128, FR], dt.bfloat16, tag="w")
                    w = w_f[:, 0:W]
                    for c in range(nck):
                        cs = slice(c * CW, (c + 1) * CW)
                        nc.vector._custom_dve(
                            wfix2, out=w_f[:, cs], in0=t_f[:, cs],
                            in1=dr_aps[c], s0=PI
                        )
                    # gd: delta = (S-I) @ w -> PSUM; wrapabs accum
                    j3_f = p_junk.tile([128, FR], dt.bfloat16, tag="junk")
                    j3 = j3_f[:, 0:W]
                    dp = p_ps.tile([128, FR], dt.float32, tag="ps")
                    for j in range(nmm):
                        sl = slice(512 * j, 512 * (j + 1))
                        nc.tensor.matmul(dp[:, sl], shiftm, w_f[:, sl],
                                         start=True, stop=True)
                    nc.vector._custom_dve(
                        wrapabs, out=j3, in0=dp[:, 0:W], s0=TWO_PI,
                        accum_out=acc[:, 32 + ui : 33 + ui],
                    )
                    # ptd: time diffs within each batch row
                    w3 = w.rearrange("p (b t) -> p b t", b=nb)
                    j2_f = p_junk.tile([128, FR], dt.bfloat16, tag="junk")
                    j2v = j2_f[:, 0:W].rearrange("p (b t) -> p b t", b=nb)
                    nc.vector._custom_dve(
                        wrapdiff,
                        out=j2v[:, :, 0 : T - 1],
                        in0=w3[:, :, 0 : T - 1],
                        in1=w3[:, :, 1:T],
                        s0=TWO_PI,
                        accum_out=acc[:, 16 + ui : 17 + ui],
                    )
                    # ip: sum |w|
                    j1_f = p_junk.tile([128, FR], dt.bfloat16, tag="junk")
                    nc.scalar.activation(
                        j1_f[:, 0:W], w, AF.Abs, accum_out=acc[:, ui : ui + 1]
                    )

                ui = 0
                for k in range(NTILES):
                    ns = SPLITS[k]
                    ub = BPC // ns
                    for h in range(ns):
                        phase1(k, h * ub, ub, ui)
                        ui += 1

            nc.sync.dma_start(out_d[:], acc[:])

    nc.compile()
    _CACHE["nc"] = nc
    return nc


# --------------------------------------------------------------------------- #
# host-side boundary terms
# --------------------------------------------------------------------------- #
def _unwrap_np(x):
    return np.abs(x - TWO_PI * np.round(x / TWO_PI))


def _host_terms(gr, gi, rr, ri):
    """gr/gi/rr/ri: [B, F, T] float32 (est_real, est_imag, ref_real, ref_imag).

    Covers: f=512 row (ip+gd+ptd), gd rows f in {0,128,256,384} (device
    computes 0 there), ptd t=0 col.
    """

    def d_of(fsl):
        pr = np.arctan2(ri[:, fsl], rr[:, fsl]).astype(np.float64)
        pg = np.arctan2(gi[:, fsl], gr[:, fsl]).astype(np.float64)
        return pr - pg

    d_rows = d_of(slice(F - 2, F))  # [B, 2, T]: f = 511, 512
    d_f0 = d_of(0)  # [B, T]
    d_t0 = np.arctan2(ri[:, :, 0], rr[:, :, 0]).astype(np.float64) - np.arctan2(
        gi[:, :, 0], gr[:, :, 0]
    ).astype(np.float64)  # [B, F]

    h_ip = _unwrap_np(d_rows[:, 1]).sum()
    h_gd = _unwrap_np(d_f0).sum() + _unwrap_np(d_rows[:, 0] - d_rows[:, 1]).sum()
    # device tile boundaries: gd rows f = 128, 256, 384
    for fb in (128, 256, 384):
        d_pair = d_of(slice(fb - 1, fb + 1))  # [B, 2, T]
        h_gd += _unwrap_np(d_pair[:, 0] - d_pair[:, 1]).sum()
    h_ptd = (
        _unwrap_np(d_t0).sum()
        + _unwrap_np(d_rows[:, 1, :-1] - d_rows[:, 1, 1:]).sum()
    )
    return h_ip + h_gd + h_ptd


# --------------------------------------------------------------------------- #
# entry point
# --------------------------------------------------------------------------- #
def kernel(**inputs) -> np.ndarray:
    from concourse.bass_utils import run_bass_kernel_spmd

    full = {
        n: np.ascontiguousarray(np.asarray(inputs[n], dtype=np.float32))
        for n in IN_NAMES
    }
    sq = {n: full[n].reshape(B, F, T) for n in IN_NAMES}

    nc = _build_bass()
    mats = shift_mats()
    if PACKED_LOAD:
        # batch-major planes (gr,gi,rr,ri); one DMA per unit on device
        packed = np.stack([sq[n] for n in IN_NAMES], axis=1)  # [B, 4, F, T]
        in_maps = [
            {
                "spec_all": np.ascontiguousarray(
                    packed[c * BPC : (c + 1) * BPC]
                ),
                "shift_mats": mats,
            }
            for c in range(NCORES)
        ]
    else:
        in_maps = [
            {
                **{
                    n: np.ascontiguousarray(sq[n][c * BPC : (c + 1) * BPC])
                    for n in IN_NAMES
                },
                "shift_mats": mats,
            }
            for c in range(NCORES)
        ]
    global LAST_RESULT
    for _attempt in range(3):
        res = run_bass_kernel_spmd(
            nc, in_maps, core_ids=list(range(NCORES)), trace=TRACE
        )
        LAST_RESULT = res
        parts = [r["partials"].astype(np.float64) for r in res.results]
        if all(np.isfinite(p).all() for p in parts):
            break
    dev_sum = float(sum(p.sum() for p in parts))

    host_sum = _host_terms(
        sq["spec_est_real"], sq["spec_est_imag"],
        sq["spec_ref_real"], sq["spec_ref_imag"],
    )
    n = float(B * F * T)
    return np.float32((dev_sum + host_sum) / n)



# revision 4
# speedup vs baseline: 1.1173x; 1.1173x over previous
"""Trainium2 Bass kernel for nn_PhaseLoss: three wrapped phase-loss terms.

loss = mean(unwrap(d)) + mean(unwrap(shift_diff_freq(d))) + mean(unwrap(shift_diff_time(d)))
with d = angle(ref) - angle(est), unwrap(x) = |x - 2pi*round(x/2pi)|.

Pure data parallel over batch (8 cores x 4 batches); per core, partition
dim = freq (4 tiles x 128 f-rows), free dim = (batch, time).

v2 design: engine-balanced pipeline (DVE was the v1 bottleneck at 71%).
  - host pre-casts the four inputs to bf16 and stacks them batch-major
    [b, 4, f, t]: HBM traffic halves and loads become plain HWDGE copies
    (no gpsimd cast-DMA), freeing the Pool engine for compute
  - per batch row b within a 128-f-row unit:
      DVE   AB = (rr*gr | ri*gi)            wide mul, bf16 2x
      Pool  C = ri*gr, D = rr*gi            (gpsimd tensor_mul)
      DVE   di = C - D
      PE    dr = I@A + I@B -> PSUM f32      (accumulating identity matmuls)
      DVE   QDIV: q = di/(dr+eps)           custom ISA (NOT-seed recip)
      ACT   t = arctan(q)
      DVE   WFIX: w = dr<0 ? t - sign(t)*pi : t
      PE    delta = (S-I)@w -> PSUM         (freq shift-diff)
      ACT   a = |delta| -> SBUF
      DVE   WRAPDIFF on time-adjacent w pairs, accum -> ptd partial
  - per unit: ACT |a - pi| accum (gd partial via min(x,2pi-x) = pi-|x-pi|),
    ACT |w| accum (ip partial)
  - host: gd partial is recovered as pi*count - accum

Host covers the boundary terms from raw inputs (~0.5% of elements):
f=512 row (all terms), gd rows f in {0, 128, 256, 384}, ptd t=0 column.
"""

import numpy as np

B, F, T = 32, 513, 512
NCORES = 8
BPC = B // NCORES  # 4
NTILES = 4  # 4 x 128 partitions = f in [0, 512)
FREE = BPC * T  # 2048
PI = float(np.pi)
TWO_PI = 2.0 * PI
IN_NAMES = ("spec_est_real", "spec_est_imag", "spec_ref_real", "spec_ref_imag")

_CACHE: dict = {}
TRACE = False
LAST_RESULT = None
REPEAT = 1

# perf toggles
SPLITS = (2, 2, 2, 2)  # batch-splits per f-tile (units of nb = BPC/ns)
POOL_CD = True         # C/D muls on GpSimd (else DVE)
GD_ACT = True          # gd wrap via 2 ACT abs passes (else DVE ISA WRAPABS)
RAW_BUFS = 4
P2_BUFS = 3
PS_BUFS = 8            # PSUM [128,512] tiles in rotation (8 banks)


# --------------------------------------------------------------------------- #
# custom DVE ops
# --------------------------------------------------------------------------- #
def _get_ops2():
    if "ops2" in _CACHE:
        return _CACHE["ops2"]
    import concourse.dve_ops as dve_ops
    from concourse.dve_ops import DveOp
    from concourse.dve_spec import (
        Bin,
        Spec,
        Src0,
        Src1,
        Zero,
        C0,
        C1,
        C2,
        _has_src1,
        lower,
        minn,
        select,
    )
    from concourse.dve_uop import AluOp, DveOpSpec
    from operator import add

    def mk(name, spec, subdim=False):
        for op in dve_ops.OPS:
            if op.name == name:
                return op
        shas = {}
        for ver in ("v3", "v4"):
            try:
                shas[ver] = DveOpSpec(
                    name=name, uops=lower(spec, ver=ver), rd1_en=_has_src1(spec)
                ).sha(ver)
            except Exception:
                pass
        op = DveOp(name, spec, subdim=subdim, uops_sha=shas)
        dve_ops.OPS.append(op)
        dve_ops._SUB_OPCODE_FOR_NAME[op.name] = (
            dve_ops._CUSTOM_DVE_ROW_BASE + len(dve_ops.OPS) - 1
        )
        dve_ops.CUSTOM_DVE_SPECS[op.name] = op.spec
        return op

    # fused q = Src0 / (Src1 + C2): signed NOT-seed reciprocal of dr (Src1)
    # times di (Src0) in one pass. The bit-NOT seed is odd in the sign bit
    # (NOT(-x) = -NOT(x) as float bits), so one seed + Newton works for both
    # signs; C2 shifts 0 -> eps. Src1 may live in PSUM.
    _g2 = Src1 + C2
    _nx2 = Bin(AluOp.BITWISE_NOT, _g2, _g2)
    _z0 = _nx2 * C0
    _z1 = _z0 * (C1 - _g2 * _z0)

    def _qd_ref(in0, in1, s0, s1, imm2):
        g = (in1.astype(np.float32) + np.float32(imm2)).astype(np.float32)
        nx = (~g.view(np.int32)).view(np.float32)
        y0 = (nx * np.float32(s0)).astype(np.float32)
        y1 = (y0 * (np.float32(s1) - g * y0)).astype(np.float32)
        return (in0.astype(np.float32) * y1).astype(np.float32)

    qdiv = mk("PL2_QDIV", Spec(body=Src0 * _z1, reference=_qd_ref))

    # w = dr<0 ? t - sign(t)*pi : t    (Src0=t, Src1=dr, C0=pi)
    _sp = select(Src0 < Zero, Zero - C0, C0)

    def _wf_ref(in0, in1, s0, s1, imm2):
        t = in0.astype(np.float32)
        sp = np.where(t < 0, -np.float32(s0), np.float32(s0)).astype(np.float32)
        return np.where(in1.astype(np.float32) < 0, t - sp, t).astype(np.float32)

    wfix2 = mk(
        "PL2_WFIX",
        Spec(body=select(Src1 < Zero, Src0 - _sp, Src0), reference=_wf_ref),
    )

    # gd fallback: out = min(|x|, 2pi-|x|), accum sum (Src0 = x = psum delta)
    _a1 = Bin(AluOp.ABSOLUTE_VALUE, Src0, Src0)

    def _wa_ref(in0, in1, s0, s1, imm2):
        a = np.abs(in0.astype(np.float32))
        b = np.minimum(a, np.float32(s0) - a).astype(np.float32)
        return b, b.reshape(b.shape[0], -1).sum(axis=-1, keepdims=True)

    wrapabs = mk(
        "PL2_WRAPABS",
        Spec(body=minn(_a1, C0 - _a1), accum=add, accum_init=Zero,
             reference=_wa_ref),
    )

    # ptd: out = min(|a-b|, 2pi-|a-b|), accum sum
    _d = Bin(AluOp.ABSOLUTE_DIFF, Src0, Src1)

    def _wd_ref(in0, in1, s0, s1, imm2):
        ad = np.abs(in0.astype(np.float32) - in1.astype(np.float32))
        b = np.minimum(ad, s0 - ad).astype(np.float32)
        return b, b.reshape(b.shape[0], -1).sum(axis=-1, keepdims=True)

    wrapdiff = mk(
        "PL2_WRAPDIFF",
        Spec(body=minn(_d, C0 - _d), accum=add, accum_init=Zero,
             reference=_wd_ref),
    )

    _CACHE["ops2"] = (wfix2, wrapabs, wrapdiff, qdiv)
    return _CACHE["ops2"]


def shift_mats() -> np.ndarray:
    """[128, 256] f32: I | (S-I)^T  (lhsT layout, out = lhsT.T @ rhs)."""
    ident = np.eye(128, dtype=np.float32)
    shift = np.zeros((128, 128), dtype=np.float32)
    # out[p] = w[p-1] - w[p] for p>=1; out[0] = 0
    shift[np.arange(127), np.arange(1, 128)] = 1.0
    shift[np.arange(1, 128), np.arange(1, 128)] = -1.0
    return np.concatenate([ident, shift], axis=1)


# --------------------------------------------------------------------------- #
# bass program (identical on all 8 cores)
# --------------------------------------------------------------------------- #
def _build_bass():
    if "nc" in _CACHE:
        return _CACHE["nc"]
    import concourse.bacc as bacc
    import concourse.tile as tile
    from concourse import mybir

    wfix2, wrapabs, wrapdiff, qdiv = _get_ops2()
    from concourse.dve_ops import RECIP_APPROX_FAST_CONSTS as RC

    dt = mybir.dt
    AF = mybir.ActivationFunctionType

    nc = bacc.Bacc("TRN2", name="phase_loss2")
    all_d = nc.dram_tensor("spec_all", [BPC, 4, F, T], dt.bfloat16,
                           kind="ExternalInput")
    mats_d = nc.dram_tensor("shift_mats", [128, 256], dt.bfloat16,
                            kind="ExternalInput")
    out_d = nc.dram_tensor("partials", [128, 48], dt.float32,
                           kind="ExternalOutput")

    with tile.TileContext(nc) as tc:
        with (
            tc.tile_pool(name="raw", bufs=RAW_BUFS) as p_raw,
            tc.tile_pool(name="prod", bufs=2) as p_prod,
            tc.tile_pool(name="pers", bufs=2) as p_pers,
            tc.tile_pool(name="p2", bufs=P2_BUFS) as p_2,
            tc.tile_pool(name="junk", bufs=2) as p_junk,
            tc.tile_pool(name="consts", bufs=1) as p_c,
            tc.tile_pool(name="accp", bufs=1) as p_acc,
            tc.psum_pool(name="ps", bufs=PS_BUFS) as p_ps,
        ):
            acc = p_acc.tile([128, 48], dt.float32, tag="acc")
            nc.vector.memset(acc[:], 0.0)
            mats = p_c.tile([128, 256], dt.bfloat16, tag="mats")
            nc.sync.dma_start(mats[:], mats_d[:, :])
            ident = mats[:, 0:128]
            shiftm = mats[:, 128:256]
            negpi = p_c.tile([128, 1], dt.float32, tag="negpi")
            nc.vector.memset(negpi[:], -PI)

            for _rep in range(REPEAT):

                def phase1(k, h0, nb, ui):
                    W = nb * T
                    # ---- load: one HWDGE DMA per unit, bf16 planes
                    # per-partition layout: [b][gr,gi,rr,ri][t]
                    UV = p_raw.tile([128, nb, 4, T], dt.bfloat16,
                                    tag=f"UV{nb}")
                    src = all_d[
                        h0 : h0 + nb, :, 128 * k : 128 * (k + 1), :
                    ].rearrange("b c f t -> f (b c) t")
                    nc.sync.dma_start(
                        UV[:].rearrange("p b c t -> p (b c) t"), src
                    )
                    # views per batch row
                    Uv = UV[:, :, 2:4, :].rearrange("p b h t -> p b (h t)")
                    Vv = UV[:, :, 0:2, :].rearrange("p b h t -> p b (h t)")
                    Ur = UV[:, :, 2, :]  # rr
                    Ui = UV[:, :, 3, :]  # ri
                    Vr = UV[:, :, 0, :]  # gr
                    Vi = UV[:, :, 1, :]  # gi

                    AB = p_prod.tile([128, nb, 2 * T], dt.bfloat16, tag="AB")
                    Cf = p_pers.tile([128, nb, T], dt.bfloat16, tag="C")
                    Df = p_pers.tile([128, nb, T], dt.bfloat16, tag="D")
                    di_f = p_2.tile([128, nb, T], dt.bfloat16, tag="di")
                    q_f = p_2.tile([128, nb, T], dt.bfloat16, tag="q")
                    t_f = p_2.tile([128, nb, T], dt.bfloat16, tag="t")
                    w_f = p_2.tile([128, nb, T], dt.bfloat16, tag="w")
                    ga_f = p_junk.tile([128, nb, T], dt.bfloat16, tag="ga")
                    j_f = p_junk.tile([128, nb, T], dt.bfloat16, tag="junk")

                    dr_ps = []
                    for bb in range(nb):
                        # products
                        nc.vector.tensor_mul(AB[:, bb], Uv[:, bb], Vv[:, bb])
                        if POOL_CD:
                            nc.gpsimd.tensor_mul(Cf[:, bb], Ui[:, bb], Vr[:, bb])
                            nc.gpsimd.tensor_mul(Df[:, bb], Ur[:, bb], Vi[:, bb])
                        else:
                            nc.vector.tensor_mul(Cf[:, bb], Ui[:, bb], Vr[:, bb])
                            nc.vector.tensor_mul(Df[:, bb], Ur[:, bb], Vi[:, bb])
                        nc.vector.tensor_sub(di_f[:, bb], Cf[:, bb], Df[:, bb])
                        # dr = A + B on PE (accumulating identity matmuls)
                        drp = p_ps.tile([128, T], dt.float32, tag="ps")
                        nc.tensor.matmul(drp[:], ident, AB[:, bb, 0:T],
                                         start=True, stop=False)
                        nc.tensor.matmul(drp[:], ident, AB[:, bb, T : 2 * T],
                                         start=False, stop=True)
                        dr_ps.append(drp)
                        # q = di / (dr + eps); eps guards dr+eps != 0
                        nc.vector._custom_dve(
                            qdiv, out=q_f[:, bb], in0=di_f[:, bb], in1=drp[:],
                            s0=RC["s0"], s1=RC["s1"], imm2=1.0000001e-6,
                        )
                        nc.scalar.activation(t_f[:, bb], q_f[:, bb], AF.Arctan)
                        nc.vector._custom_dve(
                            wfix2, out=w_f[:, bb], in0=t_f[:, bb], in1=drp[:],
                            s0=PI,
                        )
                        # gd: delta = (S-I) @ w -> PSUM
                        gdp = p_ps.tile([128, T], dt.float32, tag="ps")
                        nc.tensor.matmul(gdp[:], shiftm, w_f[:, bb],
                                         start=True, stop=True)
                        if GD_ACT:
                            nc.scalar.activation(ga_f[:, bb], gdp[:], AF.Abs)
                        else:
                            nc.vector._custom_dve(
                                wrapabs, out=ga_f[:, bb], in0=gdp[:],
                                s0=TWO_PI,
                                accum_out=acc[:, 32 + ui : 33 + ui],
                            )
                        # ptd: time diffs within this batch row
                        gb = k * BPC + h0 + bb
                        nc.vector._custom_dve(
                            wrapdiff,
                            out=j_f[:, bb, 0 : T - 1],
                            in0=w_f[:, bb, 0 : T - 1],
                            in1=w_f[:, bb, 1:T],
                            s0=TWO_PI,
                            accum_out=acc[:, 16 + gb : 17 + gb],
                        )
                    # unit-wide ACT accum passes
                    wall = w_f[:].rearrange("p b t -> p (b t)")
                    gall = ga_f[:].rearrange("p b t -> p (b t)")
                    jall = j_f[:].rearrange("p b t -> p (b t)")
                    if GD_ACT:
                        # min(x, 2pi-x) = pi - |x - pi|; host adds pi*count
                        nc.scalar.activation(
                            jall, gall, AF.Abs, bias=negpi[:],
                            accum_out=acc[:, 32 + ui : 33 + ui],
                        )
                    # ip: sum |w|
                    nc.scalar.activation(
                        gall, wall, AF.Abs, accum_out=acc[:, ui : ui + 1]
                    )

                ui = 0
                for k in range(NTILES):
                    ns = SPLITS[k]
                    ub = BPC // ns
                    for h in range(ns):
                        phase1(k, h * ub, ub, ui)
                        ui += 1

            nc.sync.dma_start(out_d[:], acc[:])

    nc.compile()
    _CACHE["nc"] = nc
    _CACHE["n_units"] = sum(SPLITS)
    return nc


# --------------------------------------------------------------------------- #
# host-side boundary terms
# --------------------------------------------------------------------------- #
def _unwrap_np(x):
    return np.abs(x - TWO_PI * np.round(x / TWO_PI))


def _host_terms(gr, gi, rr, ri):
    """gr/gi/rr/ri: [B, F, T] float32 (est_real, est_imag, ref_real, ref_imag).

    Covers: f=512 row (ip+gd+ptd), gd rows f in {0,128,256,384} (device
    computes 0 there), ptd t=0 col.
    """

    def d_of(fsl):
        pr = np.arctan2(ri[:, fsl], rr[:, fsl]).astype(np.float64)
        pg = np.arctan2(gi[:, fsl], gr[:, fsl]).astype(np.float64)
        return pr - pg

    d_rows = d_of(slice(F - 2, F))  # [B, 2, T]: f = 511, 512
    d_f0 = d_of(0)  # [B, T]
    d_t0 = np.arctan2(ri[:, :, 0], rr[:, :, 0]).astype(np.float64) - np.arctan2(
        gi[:, :, 0], gr[:, :, 0]
    ).astype(np.float64)  # [B, F]

    h_ip = _unwrap_np(d_rows[:, 1]).sum()
    h_gd = _unwrap_np(d_f0).sum() + _unwrap_np(d_rows[:, 0] - d_rows[:, 1]).sum()
    # device tile boundaries: gd rows f = 128, 256, 384
    for fb in (128, 256, 384):
        d_pair = d_of(slice(fb - 1, fb + 1))  # [B, 2, T]
        h_gd += _unwrap_np(d_pair[:, 0] - d_pair[:, 1]).sum()
    h_ptd = (
        _unwrap_np(d_t0).sum()
        + _unwrap_np(d_rows[:, 1, :-1] - d_rows[:, 1, 1:]).sum()
    )
    return h_ip + h_gd + h_ptd


# --------------------------------------------------------------------------- #
# entry point
# --------------------------------------------------------------------------- #
def kernel(**inputs) -> np.ndarray:
    import ml_dtypes
    from concourse.bass_utils import run_bass_kernel_spmd

    full = {
        n: np.ascontiguousarray(np.asarray(inputs[n], dtype=np.float32))
        for n in IN_NAMES
    }
    sq = {n: full[n].reshape(B, F, T) for n in IN_NAMES}

    nc = _build_bass()
    mats = shift_mats().astype(ml_dtypes.bfloat16)
    # batch-major bf16 planes (gr,gi,rr,ri); one HWDGE DMA per unit on device
    packed = np.stack(
        [sq[n].astype(ml_dtypes.bfloat16) for n in IN_NAMES], axis=1
    )  # [B, 4, F, T] bf16
    in_maps = [
        {
            "spec_all": np.ascontiguousarray(packed[c * BPC : (c + 1) * BPC]),
            "shift_mats": mats,
        }
        for c in range(NCORES)
    ]
    global LAST_RESULT
    for _attempt in range(3):
        res = run_bass_kernel_spmd(
            nc, in_maps, core_ids=list(range(NCORES)), trace=TRACE
        )
        LAST_RESULT = res
        parts = [r["partials"].astype(np.float64) for r in res.results]
        if all(np.isfinite(p).all() for p in parts):
            break
    n_units = _CACHE.get("n_units", sum(SPLITS))
    dev_sum = 0.0
    for p in parts:
        ip_sum = p[:, 0:16].sum()
        ptd_sum = p[:, 16:32].sum()
        if GD_ACT:
            # per unit: sum pi - |x - pi| = pi*count - accum
            gd_sum = 0.0
            ui = 0
            for k in range(NTILES):
                ns = SPLITS[k]
                W = (BPC // ns) * T
                for h in range(ns):
                    gd_sum += PI * 128 * W - p[:, 32 + ui].sum()
                    ui += 1
        else:
            gd_sum = p[:, 32:48].sum()
        dev_sum += ip_sum + ptd_sum + gd_sum

    host_sum = _host_terms(
        sq["spec_est_real"], sq["spec_est_imag"],
        sq["spec_ref_real"], sq["spec_ref_imag"],
    )
    n = float(B * F * T)
    return np.float32((dev_sum + host_sum) / n)


# revision 6
# speedup vs baseline: 1.3567x; 1.2143x over previous
"""Trainium2 Bass kernel for nn_PhaseLoss: three wrapped phase-loss terms.

loss = mean(unwrap(d)) + mean(unwrap(shift_diff_freq(d))) + mean(unwrap(shift_diff_time(d)))
with d = angle(ref) - angle(est), unwrap(x) = |x - 2pi*round(x/2pi)|.

Pure data parallel over batch (8 cores x 4 batches); per core, partition
dim = freq (4 tiles x 128 f-rows), free dim = (batch, time).

v2 design: engine-balanced pipeline (DVE was the v1 bottleneck at 71%).
  - host pre-casts the four inputs to bf16 and stacks them batch-major
    [b, 4, f, t]: HBM traffic halves and loads become plain HWDGE copies
    (no gpsimd cast-DMA), freeing the Pool engine for compute
  - per batch row b within a 128-f-row unit:
      DVE   AB = (rr*gr | ri*gi)            wide mul, bf16 2x
      Pool  C = ri*gr, D = rr*gi            (gpsimd tensor_mul)
      DVE   di = C - D
      PE    dr = I@A + I@B -> PSUM f32      (accumulating identity matmuls)
      DVE   QDIV: q = di/(dr+eps)           custom ISA (NOT-seed recip)
      ACT   t = arctan(q)
      DVE   WFIX: w = dr<0 ? t - sign(t)*pi : t
      PE    delta = (S-I)@w -> PSUM         (freq shift-diff)
      ACT   a = |delta| -> SBUF
      DVE   WRAPDIFF on time-adjacent w pairs, accum -> ptd partial
  - per unit: ACT |a - pi| accum (gd partial via min(x,2pi-x) = pi-|x-pi|),
    ACT |w| accum (ip partial)
  - host: gd partial is recovered as pi*count - accum

Host covers the boundary terms from raw inputs (~0.5% of elements):
f=512 row (all terms), gd rows f in {0, 128, 256, 384}, ptd t=0 column.
"""

import numpy as np

B, F, T = 32, 513, 512
NCORES = 8
BPC = B // NCORES  # 4
NTILES = 4  # 4 x 128 partitions = f in [0, 512)
FREE = BPC * T  # 2048
PI = float(np.pi)
TWO_PI = 2.0 * PI
IN_NAMES = ("spec_est_real", "spec_est_imag", "spec_ref_real", "spec_ref_imag")

_CACHE: dict = {}
TRACE = False
LAST_RESULT = None
REPEAT = 1

# perf toggles
SPLITS = (2, 1, 1, 2)  # batch-splits per f-tile (units of nb = BPC/ns)
POOL_CD = True         # C/D muls on GpSimd (else DVE)
GD_ACT = True          # gd wrap via 2 ACT abs passes (else DVE ISA WRAPABS)
RAW_BUFS = 3
P2_BUFS = 2
PS_BUFS = 8            # PSUM [128,512] tiles in rotation (8 banks)
ATAN_PAIR = 2          # batch rows per arctan call


# --------------------------------------------------------------------------- #
# custom DVE ops
# --------------------------------------------------------------------------- #
def _get_ops2():
    if "ops2" in _CACHE:
        return _CACHE["ops2"]
    import concourse.dve_ops as dve_ops
    from concourse.dve_ops import DveOp
    from concourse.dve_spec import (
        Bin,
        Spec,
        Src0,
        Src1,
        Zero,
        C0,
        C1,
        C2,
        _has_src1,
        lower,
        minn,
        select,
    )
    from concourse.dve_uop import AluOp, DveOpSpec
    from operator import add

    def mk(name, spec, subdim=False):
        for op in dve_ops.OPS:
            if op.name == name:
                return op
        shas = {}
        for ver in ("v3", "v4"):
            try:
                shas[ver] = DveOpSpec(
                    name=name, uops=lower(spec, ver=ver), rd1_en=_has_src1(spec)
                ).sha(ver)
            except Exception:
                pass
        op = DveOp(name, spec, subdim=subdim, uops_sha=shas)
        dve_ops.OPS.append(op)
        dve_ops._SUB_OPCODE_FOR_NAME[op.name] = (
            dve_ops._CUSTOM_DVE_ROW_BASE + len(dve_ops.OPS) - 1
        )
        dve_ops.CUSTOM_DVE_SPECS[op.name] = op.spec
        return op

    # fused q = Src0 / (Src1 + C2): signed NOT-seed reciprocal of dr (Src1)
    # times di (Src0) in one pass. The bit-NOT seed is odd in the sign bit
    # (NOT(-x) = -NOT(x) as float bits), so one seed + Newton works for both
    # signs; C2 shifts 0 -> eps. Src1 may live in PSUM.
    _g2 = Src1 + C2
    _nx2 = Bin(AluOp.BITWISE_NOT, _g2, _g2)
    _z0 = _nx2 * C0
    _z1 = _z0 * (C1 - _g2 * _z0)

    def _qd_ref(in0, in1, s0, s1, imm2):
        g = (in1.astype(np.float32) + np.float32(imm2)).astype(np.float32)
        nx = (~g.view(np.int32)).view(np.float32)
        y0 = (nx * np.float32(s0)).astype(np.float32)
        y1 = (y0 * (np.float32(s1) - g * y0)).astype(np.float32)
        return (in0.astype(np.float32) * y1).astype(np.float32)

    qdiv = mk("PL2_QDIV", Spec(body=Src0 * _z1, reference=_qd_ref))

    # w = dr<0 ? t - sign(t)*pi : t    (Src0=t, Src1=dr, C0=pi)
    _sp = select(Src0 < Zero, Zero - C0, C0)

    def _wf_ref(in0, in1, s0, s1, imm2):
        t = in0.astype(np.float32)
        sp = np.where(t < 0, -np.float32(s0), np.float32(s0)).astype(np.float32)
        return np.where(in1.astype(np.float32) < 0, t - sp, t).astype(np.float32)

    wfix2 = mk(
        "PL2_WFIX",
        Spec(body=select(Src1 < Zero, Src0 - _sp, Src0), reference=_wf_ref),
    )

    # gd fallback: out = min(|x|, 2pi-|x|), accum sum (Src0 = x = psum delta)
    _a1 = Bin(AluOp.ABSOLUTE_VALUE, Src0, Src0)

    def _wa_ref(in0, in1, s0, s1, imm2):
        a = np.abs(in0.astype(np.float32))
        b = np.minimum(a, np.float32(s0) - a).astype(np.float32)
        return b, b.reshape(b.shape[0], -1).sum(axis=-1, keepdims=True)

    wrapabs = mk(
        "PL2_WRAPABS",
        Spec(body=minn(_a1, C0 - _a1), accum=add, accum_init=Zero,
             reference=_wa_ref),
    )

    # ptd: out = min(|a-b|, 2pi-|a-b|), accum sum
    _d = Bin(AluOp.ABSOLUTE_DIFF, Src0, Src1)

    def _wd_ref(in0, in1, s0, s1, imm2):
        ad = np.abs(in0.astype(np.float32) - in1.astype(np.float32))
        b = np.minimum(ad, s0 - ad).astype(np.float32)
        return b, b.reshape(b.shape[0], -1).sum(axis=-1, keepdims=True)

    wrapdiff = mk(
        "PL2_WRAPDIFF",
        Spec(body=minn(_d, C0 - _d), accum=add, accum_init=Zero,
             reference=_wd_ref),
    )

    _CACHE["ops2"] = (wfix2, wrapabs, wrapdiff, qdiv)
    return _CACHE["ops2"]


def shift_mats() -> np.ndarray:
    """[128, 256] f32: I | (S-I)^T  (lhsT layout, out = lhsT.T @ rhs)."""
    ident = np.eye(128, dtype=np.float32)
    shift = np.zeros((128, 128), dtype=np.float32)
    # out[p] = w[p-1] - w[p] for p>=1; out[0] = 0
    shift[np.arange(127), np.arange(1, 128)] = 1.0
    shift[np.arange(1, 128), np.arange(1, 128)] = -1.0
    return np.concatenate([ident, shift], axis=1)


# --------------------------------------------------------------------------- #
# bass program (identical on all 8 cores)
# --------------------------------------------------------------------------- #
def _build_bass():
    if "nc" in _CACHE:
        return _CACHE["nc"]
    import concourse.bacc as bacc
    import concourse.tile as tile
    from concourse import mybir

    wfix2, wrapabs, wrapdiff, qdiv = _get_ops2()
    from concourse.dve_ops import RECIP_APPROX_FAST_CONSTS as RC

    dt = mybir.dt
    AF = mybir.ActivationFunctionType

    nc = bacc.Bacc("TRN2", name="phase_loss2")
    all_d = nc.dram_tensor("spec_all", [BPC, 4, F, T], dt.bfloat16,
                           kind="ExternalInput")
    mats_d = nc.dram_tensor("shift_mats", [128, 256], dt.bfloat16,
                            kind="ExternalInput")
    out_d = nc.dram_tensor("partials", [128, 48], dt.float32,
                           kind="ExternalOutput")

    with tile.TileContext(nc) as tc:
        with (
            tc.tile_pool(name="raw", bufs=RAW_BUFS) as p_raw,
            tc.tile_pool(name="prod", bufs=2) as p_prod,
            tc.tile_pool(name="pers", bufs=2) as p_pers,
            tc.tile_pool(name="p2", bufs=P2_BUFS) as p_2,
            tc.tile_pool(name="junk", bufs=2) as p_junk,
            tc.tile_pool(name="consts", bufs=1) as p_c,
            tc.tile_pool(name="accp", bufs=1) as p_acc,
            tc.psum_pool(name="ps", bufs=PS_BUFS) as p_ps,
        ):
            acc = p_acc.tile([128, 48], dt.float32, tag="acc")
            nc.vector.memset(acc[:], 0.0)
            mats = p_c.tile([128, 256], dt.bfloat16, tag="mats")
            nc.sync.dma_start(mats[:], mats_d[:, :])
            ident = mats[:, 0:128]
            shiftm = mats[:, 128:256]
            negpi = p_c.tile([128, 1], dt.float32, tag="negpi")
            nc.vector.memset(negpi[:], -PI)

            for _rep in range(REPEAT):

                def phase1(k, h0, nb, ui):
                    W = nb * T
                    # ---- load: one HWDGE DMA per unit, bf16 planes
                    # per-partition layout: [b][gr,gi,rr,ri][t]
                    UV = p_raw.tile([128, nb, 4, T], dt.bfloat16,
                                    tag=f"UV{nb}")
                    src = all_d[
                        h0 : h0 + nb, :, 128 * k : 128 * (k + 1), :
                    ].rearrange("b c f t -> f (b c) t")
                    nc.sync.dma_start(
                        UV[:].rearrange("p b c t -> p (b c) t"), src
                    )
                    # views per batch row
                    Uv = UV[:, :, 2:4, :].rearrange("p b h t -> p b (h t)")
                    Vv = UV[:, :, 0:2, :].rearrange("p b h t -> p b (h t)")
                    Ur = UV[:, :, 2, :]  # rr
                    Ui = UV[:, :, 3, :]  # ri
                    Vr = UV[:, :, 0, :]  # gr
                    Vi = UV[:, :, 1, :]  # gi

                    AB = p_prod.tile([128, nb, 2 * T], dt.bfloat16, tag="AB")
                    Cf = p_pers.tile([128, nb, T], dt.bfloat16, tag="C")
                    Df = p_pers.tile([128, nb, T], dt.bfloat16, tag="D")
                    di_f = p_2.tile([128, nb, T], dt.bfloat16, tag="di")
                    q_f = p_2.tile([128, nb, T], dt.bfloat16, tag="q")
                    t_f = p_2.tile([128, nb, T], dt.bfloat16, tag="t")
                    w_f = p_2.tile([128, nb, T], dt.bfloat16, tag="w")
                    ga_f = p_junk.tile([128, nb, T], dt.bfloat16, tag="ga")
                    j_f = p_junk.tile([128, nb, T], dt.bfloat16, tag="junk")

                    # ---- stage-ordered emission (software pipeline): each
                    # engine's queue gets a run of independent per-b ops so
                    # no engine stalls inside a cross-engine chain.
                    if POOL_CD:
                        for bb in range(nb):
                            nc.gpsimd.tensor_mul(Cf[:, bb], Ui[:, bb], Vr[:, bb])
                            nc.gpsimd.tensor_mul(Df[:, bb], Ur[:, bb], Vi[:, bb])
                    for bb in range(nb):
                        nc.vector.tensor_mul(AB[:, bb], Uv[:, bb], Vv[:, bb])
                    dr_ps = []
                    for bb in range(nb):
                        # dr = A + B on PE (accumulating identity matmuls)
                        drp = p_ps.tile([128, T], dt.float32, tag="ps")
                        nc.tensor.matmul(drp[:], ident, AB[:, bb, 0:T],
                                         start=True, stop=False)
                        nc.tensor.matmul(drp[:], ident, AB[:, bb, T : 2 * T],
                                         start=False, stop=True)
                        dr_ps.append(drp)
                    for bb in range(nb):
                        if not POOL_CD:
                            nc.vector.tensor_mul(Cf[:, bb], Ui[:, bb], Vr[:, bb])
                            nc.vector.tensor_mul(Df[:, bb], Ur[:, bb], Vi[:, bb])
                        nc.vector.tensor_sub(di_f[:, bb], Cf[:, bb], Df[:, bb])
                    for bb in range(nb):
                        # q = di / (dr + eps); eps guards dr+eps != 0
                        nc.vector._custom_dve(
                            qdiv, out=q_f[:, bb], in0=di_f[:, bb],
                            in1=dr_ps[bb][:],
                            s0=RC["s0"], s1=RC["s1"], imm2=1.0000001e-6,
                        )
                    for bb in range(0, nb, ATAN_PAIR):
                        pe = min(bb + ATAN_PAIR, nb)
                        nc.scalar.activation(
                            t_f[:, bb:pe].rearrange("p b t -> p (b t)"),
                            q_f[:, bb:pe].rearrange("p b t -> p (b t)"),
                            AF.Arctan,
                        )
                    for bb in range(nb):
                        nc.vector._custom_dve(
                            wfix2, out=w_f[:, bb], in0=t_f[:, bb],
                            in1=dr_ps[bb][:], s0=PI,
                        )
                    for bb in range(nb):
                        # gd: delta = (S-I) @ w -> PSUM
                        gdp = p_ps.tile([128, T], dt.float32, tag="ps")
                        nc.tensor.matmul(gdp[:], shiftm, w_f[:, bb],
                                         start=True, stop=True)
                        if GD_ACT:
                            nc.scalar.activation(ga_f[:, bb], gdp[:], AF.Abs)
                        else:
                            nc.vector._custom_dve(
                                wrapabs, out=ga_f[:, bb], in0=gdp[:],
                                s0=TWO_PI,
                                accum_out=acc[:, 32 + ui : 33 + ui],
                            )
                    # ptd: time diffs, whole unit in one ISA pass
                    nc.vector._custom_dve(
                        wrapdiff,
                        out=j_f[:, :, 0 : T - 1],
                        in0=w_f[:, :, 0 : T - 1],
                        in1=w_f[:, :, 1:T],
                        s0=TWO_PI,
                        accum_out=acc[:, 16 + ui : 17 + ui],
                    )
                    # unit-wide ACT accum passes
                    wall = w_f[:].rearrange("p b t -> p (b t)")
                    gall = ga_f[:].rearrange("p b t -> p (b t)")
                    jall = j_f[:].rearrange("p b t -> p (b t)")
                    if GD_ACT:
                        # min(x, 2pi-x) = pi - |x - pi|; host adds pi*count
                        nc.scalar.activation(
                            jall, gall, AF.Abs, bias=negpi[:],
                            accum_out=acc[:, 32 + ui : 33 + ui],
                        )
                    # ip: sum |w|
                    nc.scalar.activation(
                        gall, wall, AF.Abs, accum_out=acc[:, ui : ui + 1]
                    )

                ui = 0
                for k in range(NTILES):
                    ns = SPLITS[k]
                    ub = BPC // ns
                    for h in range(ns):
                        phase1(k, h * ub, ub, ui)
                        ui += 1

            nc.sync.dma_start(out_d[:], acc[:])

    nc.compile()
    _CACHE["nc"] = nc
    _CACHE["n_units"] = sum(SPLITS)
    return nc


# --------------------------------------------------------------------------- #
# host-side boundary terms
# --------------------------------------------------------------------------- #
def _unwrap_np(x):
    return np.abs(x - TWO_PI * np.round(x / TWO_PI))


def _host_terms(gr, gi, rr, ri):
    """gr/gi/rr/ri: [B, F, T] float32 (est_real, est_imag, ref_real, ref_imag).

    Covers: f=512 row (ip+gd+ptd), gd rows f in {0,128,256,384} (device
    computes 0 there), ptd t=0 col.
    """

    def d_of(fsl):
        pr = np.arctan2(ri[:, fsl], rr[:, fsl]).astype(np.float64)
        pg = np.arctan2(gi[:, fsl], gr[:, fsl]).astype(np.float64)
        return pr - pg

    d_rows = d_of(slice(F - 2, F))  # [B, 2, T]: f = 511, 512
    d_f0 = d_of(0)  # [B, T]
    d_t0 = np.arctan2(ri[:, :, 0], rr[:, :, 0]).astype(np.float64) - np.arctan2(
        gi[:, :, 0], gr[:, :, 0]
    ).astype(np.float64)  # [B, F]

    h_ip = _unwrap_np(d_rows[:, 1]).sum()
    h_gd = _unwrap_np(d_f0).sum() + _unwrap_np(d_rows[:, 0] - d_rows[:, 1]).sum()
    # device tile boundaries: gd rows f = 128, 256, 384
    for fb in (128, 256, 384):
        d_pair = d_of(slice(fb - 1, fb + 1))  # [B, 2, T]
        h_gd += _unwrap_np(d_pair[:, 0] - d_pair[:, 1]).sum()
    h_ptd = (
        _unwrap_np(d_t0).sum()
        + _unwrap_np(d_rows[:, 1, :-1] - d_rows[:, 1, 1:]).sum()
    )
    return h_ip + h_gd + h_ptd


# --------------------------------------------------------------------------- #
# entry point
# --------------------------------------------------------------------------- #
def kernel(**inputs) -> np.ndarray:
    import ml_dtypes
    from concourse.bass_utils import run_bass_kernel_spmd

    full = {
        n: np.ascontiguousarray(np.asarray(inputs[n], dtype=np.float32))
        for n in IN_NAMES
    }
    sq = {n: full[n].reshape(B, F, T) for n in IN_NAMES}

    nc = _build_bass()
    mats = shift_mats().astype(ml_dtypes.bfloat16)
    # batch-major bf16 planes (gr,gi,rr,ri); one HWDGE DMA per unit on device
    packed = np.stack(
        [sq[n].astype(ml_dtypes.bfloat16) for n in IN_NAMES], axis=1
    )  # [B, 4, F, T] bf16
    in_maps = [
        {
            "spec_all": np.ascontiguousarray(packed[c * BPC : (c + 1) * BPC]),
            "shift_mats": mats,
        }
        for c in range(NCORES)
    ]
    global LAST_RESULT
    for _attempt in range(3):
        res = run_bass_kernel_spmd(
            nc, in_maps, core_ids=list(range(NCORES)), trace=TRACE
        )
        LAST_RESULT = res
        parts = [r["partials"].astype(np.float64) for r in res.results]
        if all(np.isfinite(p).all() for p in parts):
            break
    n_units = _CACHE.get("n_units", sum(SPLITS))
    dev_sum = 0.0
    for p in parts:
        ip_sum = p[:, 0:16].sum()
        ptd_sum = p[:, 16:32].sum()
        if GD_ACT:
            # per unit: sum pi - |x - pi| = pi*count - accum
            gd_sum = 0.0
            ui = 0
            for k in range(NTILES):
                ns = SPLITS[k]
                W = (BPC // ns) * T
                for h in range(ns):
                    gd_sum += PI * 128 * W - p[:, 32 + ui].sum()
                    ui += 1
        else:
            gd_sum = p[:, 32:48].sum()
        dev_sum += ip_sum + ptd_sum + gd_sum

    host_sum = _host_terms(
        sq["spec_est_real"], sq["spec_est_imag"],
        sq["spec_ref_real"], sq["spec_ref_imag"],
    )
    n = float(B * F * T)
    return np.float32((dev_sum + host_sum) / n)
